# revision 35
# baseline (speedup 1.0000x reference)
"""Trainium2 Bass kernel for nn_AttentionLayer (B=8, H=W=64, C=256, D=128).

Strategy: data-parallel over batch B=8 across the 8 NeuronCores (attention is
independent per batch element). Per core, for its batch element's x [L=4096,
C=256]:

  phase 1: PE-transpose x tiles -> xT, project q^T,k^T [D, L] and v^T,
           then PE-transpose v^T -> v [L, D].
  phase 2 (per 512-wide Lq chunk, per 128-row Lk tile, software-pipelined):
        MM1: S^T tile    = k_tile @ q_chunk^T            (PE, fp32r)
        exp: P~^T tile   = exp(S^T tile)                 (ACT, ->fp32r)
        MM2: A~^T       += v_tile^T @ P~^T tile          (PE, accumulate)
        DVE pair-sum of two P~^T tiles, then
        MM3: denom      += ones^T @ pairsum              (PE, half the tiles)
      then per chunk: denom -> per-partition scale via 4 tiny PE transposes,
      MM4: out = A~ @ Wlast; DVE: out*scale + x; one batched output DMA.

All matmuls run in float32r (full PE rate at moving-dim>=256, ~12.7 effective
input mantissa bits measured) with fp32 PSUM accumulation. Softmax skips the
max-subtraction: logits are O(+-45) so exp stays comfortably inside fp32
range, and softmax is shift-invariant so the result matches the reference.
Measured ~283 us/core on HW (For_i marginal-time method); rel err 8.3e-4.
"""

import numpy as np

import concourse.bass as bass
import concourse.mybir as mybir
import concourse.tile as tile
from concourse import bacc
from concourse.masks import make_identity
from concourse.bass_utils import run_bass_kernel_spmd

f32 = mybir.dt.float32
f32r = mybir.dt.float32r
AF = mybir.ActivationFunctionType
ALU = mybir.AluOpType

B, H, W, C, D = 8, 64, 64, 256, 128
L = H * W            # 4096
NT = L // 128        # 32 L-tiles of 128 rows
NCHUNK = L // 512    # 8 Lq chunks of 512
CK = C // 128        # 2 C-chunks


def _emit(nc, tc, ctx, nreps=1):
    x_d = nc.declare_dram_parameter("x", [L, C], f32, isOutput=False)
    wq_d = nc.declare_dram_parameter("Wq", [C, D], f32, isOutput=False)
    wk_d = nc.declare_dram_parameter("Wk", [C, D], f32, isOutput=False)
    wv_d = nc.declare_dram_parameter("Wv", [C, D], f32, isOutput=False)
    wl_d = nc.declare_dram_parameter("Wlast", [D, C], f32, isOutput=False)
    g_d = nc.declare_dram_parameter("gamma", [1], f32, isOutput=False)
    out_d = nc.declare_dram_parameter("out", [L, C], f32, isOutput=True)

    x_tiled = x_d[:].rearrange("(t p) c -> p t c", p=128)      # [128, NT, C]
    out_tiled = out_d[:].rearrange("(t p) c -> p t c", p=128)  # [128, NT, C]

    const = ctx.enter_context(tc.tile_pool(name="const", bufs=1))
    resident = ctx.enter_context(tc.tile_pool(name="resident", bufs=1))

    # --- constants -------------------------------------------------------
    identity = const.tile([128, 128], f32)
    make_identity(nc, identity[:])
    ones_f = const.tile([128, 1], f32)
    nc.vector.memset(ones_f[:], 1.0)
    ones_r = const.tile([128, 1], f32r)
    nc.vector.tensor_copy(out=ones_r[:], in_=ones_f[:])
    id1 = const.tile([1, 1], f32)
    nc.vector.memset(id1[:], 1.0)
    gamma_sb = const.tile([128, 1], f32)
    nc.sync.dma_start(out=gamma_sb[:], in_=g_d[:].to_broadcast((128, 1)))

    # weights: lhsT chunks [C128, D] for q/k/v, [D, C] for last
    w_r = {}
    for name, wd in (("q", wq_d), ("k", wk_d), ("v", wv_d)):
        wtmp = const.tile([128, CK, D], f32, name=f"wtmp_{name}")
        nc.sync.dma_start(out=wtmp[:], in_=wd[:].rearrange("(cc p) d -> p cc d", p=128))
        wr = const.tile([128, CK, D], f32r, name=f"w_{name}")
        nc.vector.tensor_copy(out=wr[:], in_=wtmp[:])
        w_r[name] = wr
    wl_tmp = const.tile([128, C], f32)
    nc.sync.dma_start(out=wl_tmp[:], in_=wl_d[:])
    wl_r = const.tile([128, C], f32r)
    nc.vector.tensor_copy(out=wl_r[:], in_=wl_tmp[:])

    if nreps == 1:
        _emit_body(nc, tc, const, resident, x_tiled, out_tiled,
                   identity, id1, ones_r, gamma_sb, w_r, wl_r)
    else:
        # dev-harness timing build: hardware loop re-running the identical
        # body (same inputs/outputs each iteration)
        with tc.For_i(0, nreps, 1):
            _emit_body(nc, tc, const, resident, x_tiled, out_tiled,
                       identity, id1, ones_r, gamma_sb, w_r, wl_r)


def _emit_body(nc, tc, const, resident, x_tiled, out_tiled,
               identity, id1, ones_r, gamma_sb, w_r, wl_r):
    # --- resident tensors ------------------------------------------------
    x_sb = resident.tile([128, NT, C], f32, tag="x_sb")      # 32 KB/part
    for s in range(4):
        nc.sync.dma_start(
            out=x_sb[:, s * 8:(s + 1) * 8, :], in_=x_tiled[:, s * 8:(s + 1) * 8, :]
        )
    qT_sb = resident.tile([128, L], f32r, tag="qT")          # 16 KB/part
    kT_sb = resident.tile([128, L], f32r, tag="kT")          # 16 KB/part
    v_sb = resident.tile([128, NT, D], f32r, tag="v")        # 16 KB/part

    # --- phase 1: transposes + projections -------------------------------
    with (
        tc.tile_pool(name="xt", bufs=2) as xtp,
        tc.tile_pool(name="vt", bufs=2) as vtp,
        tc.tile_pool(name="ps_tr", bufs=2, space="PSUM") as ps_tr,
        tc.tile_pool(name="ps_proj", bufs=2, space="PSUM") as ps_proj,
    ):
        for c in range(NCHUNK):
            cs = slice(c * 512, (c + 1) * 512)
            # x^T for this chunk: [128, CK, 512] (C-chunk on dim1)
            xt_c = xtp.tile([128, CK, 512], f32r)
            for cc in range(CK):
                ps = ps_tr.tile([128, 512], f32, tag="tr")
                for i in range(4):
                    t = 4 * c + i
                    nc.tensor.transpose(
                        ps[:, i * 128:(i + 1) * 128],
                        x_sb[:, t, cc * 128:(cc + 1) * 128], identity[:],
                    )
                nc.vector.tensor_copy(out=xt_c[:, cc, :], in_=ps[:])
            # q^T, k^T chunks
            for name, dstT in (("q", qT_sb), ("k", kT_sb)):
                ps = ps_proj.tile([128, 512], f32, tag="proj")
                for cc in range(CK):
                    nc.tensor.matmul(
                        ps[:], w_r[name][:, cc, :], xt_c[:, cc, :],
                        start=(cc == 0), stop=(cc == CK - 1),
                    )
                nc.vector.tensor_copy(out=dstT[:, cs], in_=ps[:])
            # v^T chunk then transpose into v [L-tile, D] blocks
            ps = ps_proj.tile([128, 512], f32, tag="proj")
            for cc in range(CK):
                nc.tensor.matmul(
                    ps[:], w_r["v"][:, cc, :], xt_c[:, cc, :],
                    start=(cc == 0), stop=(cc == CK - 1),
                )
            vt_c = vtp.tile([128, 512], f32)
            nc.vector.tensor_copy(out=vt_c[:], in_=ps[:])
            ps2 = ps_tr.tile([128, 512], f32, tag="tr")
            for i in range(4):
                nc.tensor.transpose(
                    ps2[:, i * 128:(i + 1) * 128],
                    vt_c[:, i * 128:(i + 1) * 128], identity[:],
                )
            nc.vector.tensor_copy(out=v_sb[:, 4 * c:4 * c + 4, :], in_=ps2[:])

    # --- phase 2: attention ----------------------------------------------
    NG = NT // 2  # 16 exp-groups of 2 Lk tiles
    with (
        tc.tile_pool(name="pexp", bufs=12) as pexp,
        tc.tile_pool(name="psum2p", bufs=6) as psum2p,
        tc.tile_pool(name="asb", bufs=2) as asb,
        tc.tile_pool(name="osb", bufs=2) as osb,
        tc.tile_pool(name="dsb", bufs=2) as dsb,
        tc.tile_pool(name="ps_s", bufs=4, space="PSUM") as ps_s,
        tc.tile_pool(name="ps_acc", bufs=2, space="PSUM") as ps_acc,
        tc.tile_pool(name="ps_den", bufs=1, space="PSUM") as ps_den,
        tc.tile_pool(name="ps_sc", bufs=1, space="PSUM") as ps_sc_p,
    ):
        import os
        variant = os.environ.get("KVARIANT", "")
        if variant == "pefloor":
            pconst = const.tile([128, 2, 512], f32r, name="pconst")
            nc.vector.memset(pconst[:].bitcast(f32), 1.0)

        def emit_tail(c, acc, den):
            # denominator row -> free dim of partition 0, then transpose to
            # per-partition scale columns
            tall = dsb.tile([1, 512], f32, tag="tall", name="tall")
            nc.vector.tensor_copy(out=tall[:], in_=den[:])
            ps_sc = ps_sc_p.tile([128, 4], f32, tag="ps_sc", name="ps_sc")
            for m in range(4):
                nc.tensor.transpose(
                    ps_sc[:, m:m + 1], tall[0:1, m * 128:(m + 1) * 128], id1[:]
                )
            sc_raw = dsb.tile([128, 4], f32, tag="scraw", name="scraw")
            nc.vector.tensor_copy(out=sc_raw[:], in_=ps_sc[:])
            sc = dsb.tile([128, 4], f32, tag="sc", name="sc")
            nc.vector.reciprocal(out=sc[:], in_=sc_raw[:])
            nc.vector.tensor_scalar_mul(sc[:], sc[:], gamma_sb[:])

            # A~^T to SBUF (fp32r) for MM4
            a_sb = asb.tile([128, 512], f32r, tag="a_sb", name="a_sb")
            nc.vector.tensor_copy(out=a_sb[:], in_=acc[:])

            o_sb = osb.tile([128, 4, C], f32, tag="o_sb", name="o_sb")
            for m in range(4):
                t = 4 * c + m
                # MM4 output shares the s1 slot rotation (keeps PSUM at 8 banks)
                po = ps_s.tile([128, C], f32, tag="s1", name="po")
                nc.tensor.matmul(
                    po[:], a_sb[:, m * 128:(m + 1) * 128], wl_r[:],
                    start=True, stop=True,
                )
                nc.vector.scalar_tensor_tensor(
                    out=o_sb[:, m, :], in0=po[:], scalar=sc[:, m:m + 1],
                    in1=x_sb[:, t, :], op0=ALU.mult, op1=ALU.add,
                )
            nc.sync.dma_start(
                out=out_tiled[:, 4 * c:4 * c + 4, :], in_=o_sb[:]
            )

        pending_tail = None
        for c in range(NCHUNK):
            cs = slice(c * 512, (c + 1) * 512)
            acc = ps_acc.tile([128, 512], f32)
            den = ps_den.tile([1, 512], f32)

            def mm23_for(ptiles, qq, g, acc=acc, den=den):
                for j in range(2):
                    lk = 2 * g + j
                    nc.tensor.matmul(
                        acc[:], v_sb[:, lk, :], ptiles[j][:]
                        if variant != "pefloor" else ptiles[j][:, 0, :],
                        start=(lk == 0), stop=(lk == NT - 1),
                        skip_group_check=True,
                    )
                if variant != "nomm3" and qq is not None:
                    pair = g // 2
                    nc.tensor.matmul(
                        den[:], ones_r[:], qq[:],
                        start=(pair == 0), stop=(pair == NG // 2 - 1),
                        skip_group_check=True,
                    )

            pipe = []
            for g in range(NG):
                ptiles = []
                for j in range(2):
                    lk = 2 * g + j
                    s1 = ps_s.tile([128, 512], f32, tag="s1", name="s1")
                    nc.tensor.matmul(
                        s1[:], kT_sb[:, lk * 128:(lk + 1) * 128],
                        qT_sb[:, cs], start=True, stop=True,
                    )
                    if variant == "pefloor":
                        ptiles.append(pconst)
                        snk = dsb.tile([128, 2], f32, tag="snk", name="snk")
                        nc.vector.tensor_copy(out=snk[:], in_=s1[:, 0:2])
                    else:
                        p1 = pexp.tile([128, 512], f32r, tag="p1", name="p1")
                        nc.scalar.activation(out=p1[:], in_=s1[:], func=AF.Exp)
                        ptiles.append(p1)
                # reduce 4 P~ tiles to one on DVE so the denominator
                # matmul streams a quarter of the columns
                ps2 = psum2p.tile([128, 512], f32r, tag="ps2", name="ps2")
                nc.vector.tensor_tensor(
                    out=ps2[:], in0=ptiles[0][:].bitcast(f32),
                    in1=ptiles[1][:].bitcast(f32), op=ALU.add,
                )
                if g % 2 == 0:
                    prev_ps2 = ps2
                    qq = None
                else:
                    qq = psum2p.tile([128, 512], f32r, tag="qq", name="qq")
                    nc.vector.tensor_tensor(
                        out=qq[:], in0=prev_ps2[:].bitcast(f32),
                        in1=ps2[:].bitcast(f32), op=ALU.add,
                    )
                pipe.append((ptiles, qq, g))
                if len(pipe) > 3:
                    mm23_for(*pipe.pop(0))
                if g == 2 and pending_tail is not None:
                    # previous chunk's tail interleaves with this chunk's
                    # matmul stream instead of stalling the PE at the boundary
                    emit_tail(*pending_tail)
                    pending_tail = None
            while pipe:
                mm23_for(*pipe.pop(0))
            pending_tail = (c, acc, den)
        emit_tail(*pending_tail)


_NC_CACHE = {}


def _build(nreps=1):
    """Build the Bass module; nreps>1 repeats the whole body (for marginal-
    time measurement in the dev harness — grading path uses nreps=1)."""
    if nreps not in _NC_CACHE:
        from contextlib import ExitStack

        nc = bacc.Bacc("TRN2", target_bir_lowering=False)
        with tile.TileContext(nc) as tc:
            with ExitStack() as ctx:
                _emit(nc, tc, ctx, nreps=nreps)
        nc.compile()
        _NC_CACHE[nreps] = nc
    return _NC_CACHE[nreps]


def kernel(x, Wq, Wk, Wv, Wlast, gamma):
    assert x.shape == (B, H, W, C), x.shape
    nc = _build()
    xf = np.ascontiguousarray(x, dtype=np.float32).reshape(B, L, C)
    in_maps = [
        {
            "x": xf[b],
            "Wq": np.ascontiguousarray(Wq, dtype=np.float32),
            "Wk": np.ascontiguousarray(Wk, dtype=np.float32),
            "Wv": np.ascontiguousarray(Wv, dtype=np.float32),
            "Wlast": np.ascontiguousarray(Wlast, dtype=np.float32),
            "gamma": np.ascontiguousarray(gamma, dtype=np.float32),
        }
        for b in range(B)
    ]
    res = run_bass_kernel_spmd(nc, in_maps, core_ids=list(range(B)))
    out = np.stack([res.results[b]["out"] for b in range(B)], axis=0)
    return out.reshape(B, H, W, C)


# revision 37
# speedup vs baseline: 1.0574x; 1.0574x over previous
"""Trainium2 Bass kernel for nn_AttentionLayer (B=8, H=W=64, C=256, D=128).

Strategy: data-parallel over batch B=8 across the 8 NeuronCores (attention is
independent per batch element). Per core, for its batch element's x [L=4096,
C=256]:

  phase 1: PE-transpose x tiles -> xT, project q^T,k^T [D, L] and v^T,
           then PE-transpose v^T -> v [L, D].
  phase 2 (per 512-wide Lq chunk, per 128-row Lk tile, software-pipelined):
        MM1: S^T tile    = k_tile @ q_chunk^T            (PE, fp32r)
        exp: P~^T tile   = exp(S^T tile)                 (ACT, ->fp32r)
        MM2: A~^T       += v_tile^T @ P~^T tile          (PE, accumulate)
        DVE pair-sum of two P~^T tiles, then
        MM3: denom      += ones^T @ pairsum              (PE, half the tiles)
      then per chunk: denom -> per-partition scale via 4 tiny PE transposes,
      MM4: out = A~ @ Wlast; DVE: out*scale + x; one batched output DMA.

All matmuls run in float32r (full PE rate at moving-dim>=256, ~12.7 effective
input mantissa bits measured) with fp32 PSUM accumulation. Softmax skips the
max-subtraction: logits are O(+-45) so exp stays comfortably inside fp32
range, and softmax is shift-invariant so the result matches the reference.
Measured ~283 us/core on HW (For_i marginal-time method); rel err 8.3e-4.
"""

import numpy as np

import concourse.bass as bass
import concourse.mybir as mybir
import concourse.tile as tile
from concourse import bacc
from concourse.masks import make_identity
from concourse.bass_utils import run_bass_kernel_spmd

f32 = mybir.dt.float32
f32r = mybir.dt.float32r
AF = mybir.ActivationFunctionType
ALU = mybir.AluOpType

B, H, W, C, D = 8, 64, 64, 256, 128
L = H * W            # 4096
NT = L // 128        # 32 L-tiles of 128 rows
NCHUNK = L // 512    # 8 Lq chunks of 512
CK = C // 128        # 2 C-chunks


def _emit(nc, tc, ctx, nreps=1):
    x_d = nc.declare_dram_parameter("x", [L, C], f32, isOutput=False)
    wq_d = nc.declare_dram_parameter("Wq", [C, D], f32, isOutput=False)
    wk_d = nc.declare_dram_parameter("Wk", [C, D], f32, isOutput=False)
    wv_d = nc.declare_dram_parameter("Wv", [C, D], f32, isOutput=False)
    wl_d = nc.declare_dram_parameter("Wlast", [D, C], f32, isOutput=False)
    g_d = nc.declare_dram_parameter("gamma", [1], f32, isOutput=False)
    out_d = nc.declare_dram_parameter("out", [L, C], f32, isOutput=True)

    x_tiled = x_d[:].rearrange("(t p) c -> p t c", p=128)      # [128, NT, C]
    out_tiled = out_d[:].rearrange("(t p) c -> p t c", p=128)  # [128, NT, C]

    const = ctx.enter_context(tc.tile_pool(name="const", bufs=1))
    resident = ctx.enter_context(tc.tile_pool(name="resident", bufs=1))

    # --- constants -------------------------------------------------------
    identity = const.tile([128, 128], f32)
    make_identity(nc, identity[:])
    ones_f = const.tile([128, 1], f32)
    nc.vector.memset(ones_f[:], 1.0)
    ones_r = const.tile([128, 1], f32r)
    nc.vector.tensor_copy(out=ones_r[:], in_=ones_f[:])
    id1 = const.tile([1, 1], f32)
    nc.vector.memset(id1[:], 1.0)
    gamma_sb = const.tile([128, 1], f32)
    nc.sync.dma_start(out=gamma_sb[:], in_=g_d[:].to_broadcast((128, 1)))

    # weights: lhsT chunks [C128, D] for q/k/v, [D, C] for last
    w_r = {}
    for name, wd in (("q", wq_d), ("k", wk_d), ("v", wv_d)):
        wtmp = const.tile([128, CK, D], f32, name=f"wtmp_{name}")
        nc.sync.dma_start(out=wtmp[:], in_=wd[:].rearrange("(cc p) d -> p cc d", p=128))
        wr = const.tile([128, CK, D], f32r, name=f"w_{name}")
        nc.vector.tensor_copy(out=wr[:], in_=wtmp[:])
        w_r[name] = wr
    wl_tmp = const.tile([128, C], f32)
    nc.sync.dma_start(out=wl_tmp[:], in_=wl_d[:])
    wl_r = const.tile([128, C], f32r)
    nc.vector.tensor_copy(out=wl_r[:], in_=wl_tmp[:])

    if nreps == 1:
        _emit_body(nc, tc, const, resident, x_tiled, out_tiled,
                   identity, id1, ones_r, gamma_sb, w_r, wl_r)
    else:
        # dev-harness timing build: hardware loop re-running the identical
        # body (same inputs/outputs each iteration)
        with tc.For_i(0, nreps, 1):
            _emit_body(nc, tc, const, resident, x_tiled, out_tiled,
                       identity, id1, ones_r, gamma_sb, w_r, wl_r)


def _emit_body(nc, tc, const, resident, x_tiled, out_tiled,
               identity, id1, ones_r, gamma_sb, w_r, wl_r):
    # --- resident tensors ------------------------------------------------
    x_sb = resident.tile([128, NT, C], f32, tag="x_sb")      # 32 KB/part
    for s in range(4):
        nc.sync.dma_start(
            out=x_sb[:, s * 8:(s + 1) * 8, :], in_=x_tiled[:, s * 8:(s + 1) * 8, :]
        )
    qT_sb = resident.tile([128, L], f32r, tag="qT")          # 16 KB/part
    kT_sb = resident.tile([128, L], f32r, tag="kT")          # 16 KB/part
    v_sb = resident.tile([128, NT, D], f32r, tag="v")        # 16 KB/part

    # --- phase 1: transposes + projections -------------------------------
    import os
    _p1b = 3 if os.environ.get("KP1", "1") == "1" else 2
    with (
        tc.tile_pool(name="xt", bufs=_p1b) as xtp,
        tc.tile_pool(name="vt", bufs=_p1b) as vtp,
        tc.tile_pool(name="ps_tr", bufs=_p1b, space="PSUM") as ps_tr,
        tc.tile_pool(name="ps_proj", bufs=_p1b, space="PSUM") as ps_proj,
    ):
        for c in range(NCHUNK):
            cs = slice(c * 512, (c + 1) * 512)
            # x^T for this chunk: [128, CK, 512] (C-chunk on dim1)
            xt_c = xtp.tile([128, CK, 512], f32r)
            for cc in range(CK):
                ps = ps_tr.tile([128, 512], f32, tag="tr")
                for i in range(4):
                    t = 4 * c + i
                    nc.tensor.transpose(
                        ps[:, i * 128:(i + 1) * 128],
                        x_sb[:, t, cc * 128:(cc + 1) * 128], identity[:],
                    )
                nc.vector.tensor_copy(out=xt_c[:, cc, :], in_=ps[:])
            # q^T, k^T chunks
            for name, dstT in (("q", qT_sb), ("k", kT_sb)):
                ps = ps_proj.tile([128, 512], f32, tag="proj")
                for cc in range(CK):
                    nc.tensor.matmul(
                        ps[:], w_r[name][:, cc, :], xt_c[:, cc, :],
                        start=(cc == 0), stop=(cc == CK - 1),
                    )
                nc.vector.tensor_copy(out=dstT[:, cs], in_=ps[:])
            # v^T chunk then transpose into v [L-tile, D] blocks
            ps = ps_proj.tile([128, 512], f32, tag="proj")
            for cc in range(CK):
                nc.tensor.matmul(
                    ps[:], w_r["v"][:, cc, :], xt_c[:, cc, :],
                    start=(cc == 0), stop=(cc == CK - 1),
                )
            vt_c = vtp.tile([128, 512], f32)
            nc.vector.tensor_copy(out=vt_c[:], in_=ps[:])
            ps2 = ps_tr.tile([128, 512], f32, tag="tr")
            for i in range(4):
                nc.tensor.transpose(
                    ps2[:, i * 128:(i + 1) * 128],
                    vt_c[:, i * 128:(i + 1) * 128], identity[:],
                )
            nc.vector.tensor_copy(out=v_sb[:, 4 * c:4 * c + 4, :], in_=ps2[:])

    # --- phase 2: attention ----------------------------------------------
    NG = NT // 2  # 16 exp-groups of 2 Lk tiles
    with (
        tc.tile_pool(name="pexp", bufs=12) as pexp,
        tc.tile_pool(name="psum2p", bufs=6) as psum2p,
        tc.tile_pool(name="asb", bufs=2) as asb,
        tc.tile_pool(name="osb", bufs=2) as osb,
        tc.tile_pool(name="dsb", bufs=2) as dsb,
        tc.tile_pool(name="ps_s", bufs=4, space="PSUM") as ps_s,
        tc.tile_pool(name="ps_acc", bufs=2, space="PSUM") as ps_acc,
        tc.tile_pool(name="ps_den", bufs=1, space="PSUM") as ps_den,
        tc.tile_pool(name="ps_sc", bufs=1, space="PSUM") as ps_sc_p,
    ):
        import os
        variant = os.environ.get("KVARIANT", "")
        if variant == "pefloor":
            pconst = const.tile([128, 2, 512], f32r, name="pconst")
            nc.vector.memset(pconst[:].bitcast(f32), 1.0)

        def emit_tail(c, acc, den):
            # denominator row -> free dim of partition 0, then transpose to
            # per-partition scale columns
            tall = dsb.tile([1, 512], f32, tag="tall", name="tall")
            nc.vector.tensor_copy(out=tall[:], in_=den[:])
            ps_sc = ps_sc_p.tile([128, 4], f32, tag="ps_sc", name="ps_sc")
            for m in range(4):
                nc.tensor.transpose(
                    ps_sc[:, m:m + 1], tall[0:1, m * 128:(m + 1) * 128], id1[:]
                )
            sc_raw = dsb.tile([128, 4], f32, tag="scraw", name="scraw")
            nc.vector.tensor_copy(out=sc_raw[:], in_=ps_sc[:])
            sc = dsb.tile([128, 4], f32, tag="sc", name="sc")
            nc.vector.reciprocal(out=sc[:], in_=sc_raw[:])
            nc.vector.tensor_scalar_mul(sc[:], sc[:], gamma_sb[:])

            # A~^T to SBUF (fp32r) for MM4
            a_sb = asb.tile([128, 512], f32r, tag="a_sb", name="a_sb")
            nc.vector.tensor_copy(out=a_sb[:], in_=acc[:])

            o_sb = osb.tile([128, 4, C], f32, tag="o_sb", name="o_sb")
            for m in range(4):
                t = 4 * c + m
                # MM4 output shares the s1 slot rotation (keeps PSUM at 8 banks)
                po = ps_s.tile([128, C], f32, tag="s1", name="po")
                nc.tensor.matmul(
                    po[:], a_sb[:, m * 128:(m + 1) * 128], wl_r[:],
                    start=True, stop=True,
                )
                nc.vector.scalar_tensor_tensor(
                    out=o_sb[:, m, :], in0=po[:], scalar=sc[:, m:m + 1],
                    in1=x_sb[:, t, :], op0=ALU.mult, op1=ALU.add,
                )
            nc.sync.dma_start(
                out=out_tiled[:, 4 * c:4 * c + 4, :], in_=o_sb[:]
            )

        pending_tail = None
        for c in range(NCHUNK):
            cs = slice(c * 512, (c + 1) * 512)
            acc = ps_acc.tile([128, 512], f32)
            den = ps_den.tile([1, 512], f32)

            def mm23_for(ptiles, qq, g, acc=acc, den=den):
                for j in range(2):
                    lk = 2 * g + j
                    nc.tensor.matmul(
                        acc[:], v_sb[:, lk, :], ptiles[j][:]
                        if variant != "pefloor" else ptiles[j][:, 0, :],
                        start=(lk == 0), stop=(lk == NT - 1),
                        skip_group_check=True,
                    )
                if variant != "nomm3" and qq is not None:
                    if os.environ.get("KQUAD", "1") != "1":
                        first, last = (g == 0), (g == NG - 1)
                    else:
                        first = (g // 2 == 0)
                        last = (g // 2 == NG // 2 - 1)
                    nc.tensor.matmul(
                        den[:], ones_r[:], qq[:],
                        start=first, stop=last,
                        skip_group_check=True,
                    )

            pipe = []
            for g in range(NG):
                ptiles = []
                for j in range(2):
                    lk = 2 * g + j
                    s1 = ps_s.tile([128, 512], f32, tag="s1", name="s1")
                    nc.tensor.matmul(
                        s1[:], kT_sb[:, lk * 128:(lk + 1) * 128],
                        qT_sb[:, cs], start=True, stop=True,
                    )
                    if variant == "pefloor":
                        ptiles.append(pconst)
                        snk = dsb.tile([128, 2], f32, tag="snk", name="snk")
                        nc.vector.tensor_copy(out=snk[:], in_=s1[:, 0:2])
                    else:
                        p1 = pexp.tile([128, 512], f32r, tag="p1", name="p1")
                        nc.scalar.activation(out=p1[:], in_=s1[:], func=AF.Exp)
                        ptiles.append(p1)
                # reduce 4 P~ tiles to one on DVE so the denominator
                # matmul streams a quarter of the columns
                ps2 = psum2p.tile([128, 512], f32r, tag="ps2", name="ps2")
                nc.vector.tensor_tensor(
                    out=ps2[:], in0=ptiles[0][:].bitcast(f32),
                    in1=ptiles[1][:].bitcast(f32), op=ALU.add,
                )
                if os.environ.get("KQUAD", "1") != "1":
                    qq = ps2  # v7 behaviour: one denominator MM per group
                elif g % 2 == 0:
                    prev_ps2 = ps2
                    qq = None
                else:
                    qq = psum2p.tile([128, 512], f32r, tag="qq", name="qq")
                    nc.vector.tensor_tensor(
                        out=qq[:], in0=prev_ps2[:].bitcast(f32),
                        in1=ps2[:].bitcast(f32), op=ALU.add,
                    )
                pipe.append((ptiles, qq, g))
                if len(pipe) > 3:
                    mm23_for(*pipe.pop(0))
                if g == 2 and pending_tail is not None:
                    # previous chunk's tail interleaves with this chunk's
                    # matmul stream instead of stalling the PE at the boundary
                    emit_tail(*pending_tail)
                    pending_tail = None
            while pipe:
                mm23_for(*pipe.pop(0))
            pending_tail = (c, acc, den)
        emit_tail(*pending_tail)


_NC_CACHE = {}


def _build(nreps=1):
    """Build the Bass module; nreps>1 repeats the whole body (for marginal-
    time measurement in the dev harness — grading path uses nreps=1)."""
    if nreps not in _NC_CACHE:
        from contextlib import ExitStack

        nc = bacc.Bacc("TRN2", target_bir_lowering=False)
        with tile.TileContext(nc) as tc:
            with ExitStack() as ctx:
                _emit(nc, tc, ctx, nreps=nreps)
        nc.compile()
        _NC_CACHE[nreps] = nc
    return _NC_CACHE[nreps]


def kernel(x, Wq, Wk, Wv, Wlast, gamma):
    assert x.shape == (B, H, W, C), x.shape
    nc = _build()
    xf = np.ascontiguousarray(x, dtype=np.float32).reshape(B, L, C)
    in_maps = [
        {
            "x": xf[b],
            "Wq": np.ascontiguousarray(Wq, dtype=np.float32),
            "Wk": np.ascontiguousarray(Wk, dtype=np.float32),
            "Wv": np.ascontiguousarray(Wv, dtype=np.float32),
            "Wlast": np.ascontiguousarray(Wlast, dtype=np.float32),
            "gamma": np.ascontiguousarray(gamma, dtype=np.float32),
        }
        for b in range(B)
    ]
    res = run_bass_kernel_spmd(nc, in_maps, core_ids=list(range(B)))
    out = np.stack([res.results[b]["out"] for b in range(B)], axis=0)
    return out.reshape(B, H, W, C)
